# revision 8
# baseline (speedup 1.0000x reference)
#!/usr/bin/env python3
"""Gumbel top-k sampler kernel for Trainium2 (8 NeuronCores, data-parallel).

Computes, per batch row b (B=32, L=8192, D=128, K=512):
    z = probs/T + gumbel + mask*(-1e4)           (gumbel: fixed key -> constant)
    top-K selection of softmax(z) == top-K of z  (softmax is monotonic)
    sampled_reps = reps * onehot(topK set)       (straight-through value)
    ind = indices of top-K, descending by value

Sharding: data-parallel over B across 8 cores (4 rows/core).

Device algorithm per core:
  - z in [128,256] layout (row r on partitions 32r..32r+31, 256 elems each)
  - threshold t_r (512th largest) via 32-step count bisection
    (counts: tensor_scalar is_ge accum -> per-partition, matmul block-diag
     -> per-row)
  - sampled_reps: stream reps tiles, multiply by per-l-chunk indicator
    scalars (indicator transposed to l-on-partition layout via TensorE)
  - ind: compact selected l's via prefix-scan + local_scatter + matmul
    row-combine; gather their z values (ap_gather); rank each selected
    element by counting greater values (tensor_scalar is_gt accum, 16
    passes); scatter l's by rank via local_scatter; emit int32
"""
import sys
import os

sys.path.insert(0, "/opt/trn_rl_repo")

import numpy as np

B, L, D, K = 32, 8192, 128, 512
NCORES = 8
RPC = B // NCORES          # rows per core = 4
CH = 32                    # partitions (chunks) per row in z1 layout
F = L // CH                # 256 free elems per partition
NITER = 32                 # bisection iterations
LO0, HI0 = -200.0, 200.0   # bisection bracket (z range is ~[-10050, 80])

_compiled = None


def _build_program():
    import concourse.bacc as bacc
    import concourse.mybir as mybir
    import concourse.tile as tile

    dt = mybir.dt
    Alu = mybir.AluOpType

    nc = bacc.Bacc("TRN2", target_bir_lowering=False, debug=False)

    probs = nc.dram_tensor("probs", [RPC, L], dt.float32, kind="ExternalInput")
    maskt = nc.dram_tensor("mask", [RPC, L], dt.float32, kind="ExternalInput")
    gumb = nc.dram_tensor("gumbel", [RPC, L], dt.float32, kind="ExternalInput")
    reps = nc.dram_tensor("reps", [RPC, L, D], dt.float32, kind="ExternalInput")
    b1c = nc.dram_tensor("b1c", [128, RPC], dt.float32, kind="ExternalInput")
    b2c = nc.dram_tensor("b2c", [RPC, 128], dt.float32, kind="ExternalInput")
    tric = nc.dram_tensor("tric", [128, 128], dt.float32, kind="ExternalInput")
    onesc = nc.dram_tensor("onesc", [128, 1], dt.float32, kind="ExternalInput")
    ones1r = nc.dram_tensor("ones1r", [64, 128], dt.float32, kind="ExternalInput")
    identc = nc.dram_tensor("identc", [128, 128], dt.float32, kind="ExternalInput")
    lg16c = nc.dram_tensor("lg16c", [128, 256], dt.uint16, kind="ExternalInput")
    roff4c = nc.dram_tensor("roff4c", [RPC, 1], dt.float32, kind="ExternalInput")
    roff16c = nc.dram_tensor("roff16c", [128, 16], dt.float32, kind="ExternalInput")

    out_reps = nc.dram_tensor(
        "out_reps", [RPC, L, D], dt.float32, kind="ExternalOutput"
    )
    out_ind = nc.dram_tensor("out_ind", [RPC, K], dt.int32, kind="ExternalOutput")
    out_dbg = nc.dram_tensor("out_dbg", [RPC, 8], dt.float32, kind="ExternalOutput")

    with tile.TileContext(nc) as tc:
        with (
            tc.tile_pool(name="persist", bufs=1) as pp,
            tc.tile_pool(name="small", bufs=2) as sp,
            tc.tile_pool(name="psum", bufs=1, space="PSUM") as ps,
            tc.tile_pool(name="psbig", bufs=1, space="PSUM") as psb,
            tc.tile_pool(name="repin", bufs=3) as rin,
            tc.tile_pool(name="repout", bufs=3) as rout,
            tc.tile_pool(name="dram", bufs=1, space="DRAM") as dp,
        ):
            # ---------------- constants to SBUF ----------------
            b1 = pp.tile([128, RPC], dt.float32)
            b2 = pp.tile([RPC, 128], dt.float32)
            tri = pp.tile([128, 128], dt.float32)
            ones_col = pp.tile([128, 1], dt.float32)
            ones_row = pp.tile([64, 128], dt.float32)
            nc.sync.dma_start(b1[:], b1c[:])
            nc.sync.dma_start(b2[:], b2c[:])
            nc.sync.dma_start(tri[:], tric[:])
            nc.sync.dma_start(ones_col[:], onesc[:])
            nc.sync.dma_start(ones_row[:], ones1r[:])
            ident128 = pp.tile([128, 128], dt.float32)
            nc.sync.dma_start(ident128[:], identc[:])

            # ---------------- z in [128, 256] layout ----------------
            p1 = pp.tile([128, F], dt.float32)
            g1 = pp.tile([128, F], dt.float32)
            m1 = pp.tile([128, F], dt.float32)
            nc.sync.dma_start(p1[:], probs.rearrange("r (c f) -> (r c) f", c=CH)[:])
            nc.sync.dma_start(g1[:], gumb.rearrange("r (c f) -> (r c) f", c=CH)[:])
            nc.sync.dma_start(m1[:], maskt.rearrange("r (c f) -> (r c) f", c=CH)[:])
            zf = pp.tile([128, F], dt.float32)
            nc.vector.scalar_tensor_tensor(
                zf[:], p1[:], 10.0, g1[:], Alu.mult, Alu.add
            )
            nc.vector.scalar_tensor_tensor(
                zf[:], m1[:], -10000.0, zf[:], Alu.mult, Alu.add
            )

            # ---------------- bisection for per-row threshold ----------------
            lo = pp.tile([RPC, 1], dt.float32)
            hi = pp.tile([RPC, 1], dt.float32)
            nc.vector.memset(lo[:], LO0)
            nc.vector.memset(hi[:], HI0)
            junk = pp.tile([128, F], dt.float32)
            for _ in range(NITER):
                mid = sp.tile([RPC, 1], dt.float32, tag="mid")
                nc.vector.tensor_tensor(mid[:], lo[:], hi[:], Alu.add)
                nc.vector.tensor_scalar_mul(mid[:], mid[:], 0.5)
                tpp_ps = ps.tile([128, 1], dt.float32, tag="tpp")
                nc.tensor.matmul(tpp_ps[:], b2[:], mid[:])
                tpp = sp.tile([128, 1], dt.float32, tag="tpps")
                nc.vector.tensor_copy(tpp[:], tpp_ps[:])
                cpp = sp.tile([128, 1], dt.float32, tag="cpp")
                nc.vector.tensor_scalar(
                    junk[:], zf[:], tpp[:], None, Alu.is_ge, Alu.add,
                    accum_out=cpp[:],
                )
                rc_ps = ps.tile([RPC, 1], dt.float32, tag="rc")
                nc.tensor.matmul(rc_ps[:], b1[:], cpp[:])
                ge = sp.tile([RPC, 1], dt.uint8, tag="ge")
                nc.vector.tensor_scalar(
                    ge[:], rc_ps[:], float(K), None, Alu.is_ge
                )
                lo2 = sp.tile([RPC, 1], dt.float32, tag="lo2")
                hi2 = sp.tile([RPC, 1], dt.float32, tag="hi2")
                nc.vector.select(lo2[:], ge[:], mid[:], lo[:])
                nc.vector.select(hi2[:], ge[:], hi[:], mid[:])
                nc.vector.tensor_copy(lo[:], lo2[:])
                nc.vector.tensor_copy(hi[:], hi2[:])

            # final indicator + per-partition counts at t* = lo
            tpp_ps = ps.tile([128, 1], dt.float32, tag="tpp")
            nc.tensor.matmul(tpp_ps[:], b2[:], lo[:])
            tstar = pp.tile([128, 1], dt.float32)
            nc.vector.tensor_copy(tstar[:], tpp_ps[:])
            ind1 = pp.tile([128, F], dt.float32)
            cppf = pp.tile([128, 1], dt.float32)
            nc.vector.tensor_scalar(
                ind1[:], zf[:], tstar[:], None, Alu.is_ge, Alu.add,
                accum_out=cppf[:],
            )
            # debug: per-row count at t*
            rcf_ps = ps.tile([RPC, 1], dt.float32, tag="rc")
            nc.tensor.matmul(rcf_ps[:], b1[:], cppf[:])
            dbg = pp.tile([RPC, 8], dt.float32)
            nc.vector.memset(dbg[:], 0.0)
            nc.vector.tensor_copy(dbg[:, 0:1], rcf_ps[:])
            nc.vector.tensor_copy(dbg[:, 1:2], lo[:])
            nc.sync.dma_start(out_dbg[:], dbg[:])

            # ---------------- indicator transpose: indT[p, r*64+col] --------
            # indT[p, r*64 + 2c + h] = ind of row r at l = 256c + 128h + p
            indT = pp.tile([128, RPC * 64], dt.float32)
            for h in range(2):
                tp_ps = psb.tile([128, 128], dt.float32, tag="tps")
                nc.tensor.transpose(
                    tp_ps[:], ind1[:, 128 * h:128 * h + 128], ident128[:]
                )
                # psum col (32r + c) -> indT col (64r + 2c + h)
                dst = indT.rearrange("p (r c) -> p r c", r=RPC)[
                    :, :, h::2
                ]
                nc.vector.tensor_copy(dst, tp_ps.rearrange("p (r c) -> p r c", r=RPC)[:])

            # ---------------- bulk: out_reps = reps * indicator -------------
            # tile: [128 part = l%128, 8 l-chunks x 128 d]
            TL = 8  # l-chunks (of 128) per tile
            reps_v = reps.rearrange("r (u p) d -> r p u d", p=128)
            oreps_v = out_reps.rearrange("r (u p) d -> r p u d", p=128)
            for r in range(RPC):
                for i in range(64 // TL):
                    tin = rin.tile([128, TL * D], dt.float32, tag="tin")
                    nc.sync.dma_start(
                        tin.rearrange("p (u d) -> p u d", u=TL)[:],
                        reps_v[r, :, TL * i:TL * i + TL, :],
                    )
                    tout = rout.tile([128, TL * D], dt.float32, tag="tout")
                    for j in range(TL):
                        col = r * 64 + TL * i + j
                        nc.vector.tensor_scalar(
                            tout[:, D * j:D * j + D],
                            tin[:, D * j:D * j + D],
                            indT[:, col:col + 1],
                            None,
                            Alu.mult,
                        )
                    nc.sync.dma_start(
                        oreps_v[r, :, TL * i:TL * i + TL, :],
                        tout.rearrange("p (u d) -> p u d", u=TL)[:],
                    )

            # ---------------- compaction of selected l's --------------------
            scan1 = pp.tile([128, F], dt.float32)
            nc.vector.tensor_tensor_scan(
                scan1[:], ind1[:], ind1[:], 0.0, Alu.add, Alu.bypass
            )
            excl_ps = ps.tile([128, 1], dt.float32, tag="excl")
            nc.tensor.matmul(excl_ps[:], tri[:], cppf[:])
            excl = pp.tile([128, 1], dt.float32)
            nc.vector.tensor_copy(excl[:], excl_ps[:])
            slotp1 = pp.tile([128, F], dt.float32)
            nc.vector.scalar_tensor_tensor(
                slotp1[:], scan1[:], excl[:], ind1[:], Alu.add, Alu.mult
            )
            slot = pp.tile([128, F], dt.float32)
            nc.vector.tensor_scalar(slot[:], slotp1[:], 1.0, None, Alu.subtract)
            idx16 = pp.tile([128, F], dt.int16)
            nc.vector.tensor_copy(idx16[:], slot[:])
            lg16 = pp.tile([128, F], dt.uint16)
            nc.sync.dma_start(lg16[:], lg16c[:])
            ls_out = pp.tile([128, K], dt.uint16)
            nc.gpsimd.local_scatter(
                ls_out[:], lg16[:], idx16[:], channels=128, num_elems=K,
                num_idxs=F,
            )
            lsf = pp.tile([128, K], dt.float32)
            nc.vector.tensor_copy(lsf[:], ls_out[:])
            lc_ps = psb.tile([RPC, K], dt.float32, tag="lc")
            nc.tensor.matmul(lc_ps[:], b1[:], lsf[:])
            # local l = global l - 8192*r
            roff4 = pp.tile([RPC, 1], dt.float32)
            nc.sync.dma_start(roff4[:], roff4c[:])
            lloc = pp.tile([RPC, K], dt.float32)
            nc.vector.tensor_scalar(
                lloc[:], lc_ps[:], roff4[:], None, Alu.subtract
            )

            # ---------------- gather selected z values ----------------------
            zdram = dp.tile([RPC, L], dt.float32)
            nc.sync.dma_start(
                zdram.rearrange("r (c f) -> (r c) f", c=CH)[:], zf[:]
            )
            ldram = dp.tile([RPC, K], dt.float32)
            nc.sync.dma_start(ldram[:], lloc[:])
            z64 = pp.tile([64, L], dt.float32)
            idxg_f = pp.tile([64, K // 16], dt.float32)
            for r in range(RPC):
                nc.sync.dma_start(
                    z64[16 * r:16 * r + 16, :],
                    zdram[r][None, :].broadcast_to([16, L]),
                )
                nc.sync.dma_start(
                    idxg_f[16 * r:16 * r + 16, :],
                    ldram[r].rearrange("(f p) -> p f", p=16),
                )
            idxg = pp.tile([64, K // 16], dt.int16)
            nc.vector.tensor_copy(idxg[:], idxg_f[:])
            sg = pp.tile([64, K], dt.float32)
            nc.gpsimd.ap_gather(
                sg[:], z64[:], idxg[:], channels=64, num_elems=L, d=1,
                num_idxs=K,
            )

            # ---------------- ranks via counting ----------------------------
            # sdram[r, k] = z value of k-th selected (ascending l) of row r
            sdram = dp.tile([RPC, K], dt.float32)
            for r in range(RPC):
                nc.sync.dma_start(sdram[r][None, :], sg[16 * r:16 * r + 1, :])
            # col-major remaps [4, 512] -> [128, 16]: x128[p, 4r+q] = x[r, 128q+p]
            s128 = pp.tile([128, 16], dt.float32)
            l128 = pp.tile([128, 16], dt.float32)
            for r in range(RPC):
                nc.sync.dma_start(
                    s128[:, 4 * r:4 * r + 4],
                    sdram[r].rearrange("(q p) -> p q", p=128),
                )
                nc.sync.dma_start(
                    l128[:, 4 * r:4 * r + 4],
                    ldram[r].rearrange("(q p) -> p q", p=128),
                )
            junk2 = pp.tile([128, K], dt.float32)
            ranks = pp.tile([128, 16], dt.float32)
            for r in range(RPC):
                srep = sp.tile([128, K], dt.float32, tag="sreps")
                nc.sync.dma_start(
                    srep[:], sdram[r][None, :].broadcast_to([128, K])
                )
                for q in range(4):
                    c = 4 * r + q
                    nc.vector.tensor_scalar(
                        junk2[:], srep[:], s128[:, c:c + 1], None,
                        Alu.is_gt, Alu.add, accum_out=ranks[:, c:c + 1],
                    )

            # ---------------- scatter ind by rank ---------------------------
            lu16 = pp.tile([128, 16], dt.uint16)
            nc.vector.tensor_copy(lu16[:], l128[:])
            roff16 = pp.tile([128, 16], dt.float32)
            nc.sync.dma_start(roff16[:], roff16c[:])
            radd = pp.tile([128, 16], dt.float32)
            nc.vector.tensor_tensor(radd[:], ranks[:], roff16[:], Alu.add)
            ri16 = pp.tile([128, 16], dt.int16)
            nc.vector.tensor_copy(ri16[:], radd[:])
            fin = pp.tile([128, 2 * K], dt.uint16)
            fin2 = pp.tile([128, 2 * K], dt.uint16)
            nc.gpsimd.local_scatter(
                fin[:], lu16[:, 0:8], ri16[:, 0:8], channels=128,
                num_elems=2 * K, num_idxs=8,
            )
            nc.gpsimd.local_scatter(
                fin2[:], lu16[:, 8:16], ri16[:, 8:16], channels=128,
                num_elems=2 * K, num_idxs=8,
            )
            for half, src in enumerate([fin, fin2]):
                srcf = sp.tile([128, 2 * K], dt.float32, tag="finf")
                nc.vector.tensor_copy(srcf[:], src[:])
                for j in range(2):
                    indp = ps.tile([1, K], dt.float32, tag="indp")
                    nc.tensor.matmul(indp[:], ones_col[:], srcf[:, K * j:K * j + K])
                    indi = sp.tile([1, K], dt.int32, tag="indi")
                    nc.vector.tensor_copy(indi[:], indp[:])
                    r = 2 * half + j
                    nc.sync.dma_start(out_ind[r][None, :], indi[:])

    nc.finalize()
    return nc


def _get_compiled():
    global _compiled
    if _compiled is None:
        _compiled = _build_program()
    return _compiled


def _gumbel_const():
    # jax.random with a fixed key is a pure constant; compute on host CPU
    import jax

    with jax.default_device(jax.devices("cpu")[0]):
        gkey = jax.random.key(42)
        U = np.asarray(
            jax.random.uniform(gkey, (B, L), dtype=np.float32)
        ).astype(np.float32)
    eps = np.float32(1e-20)
    return -np.log(-np.log(U + eps) + eps).astype(np.float32)


def _consts():
    b1 = np.zeros((128, RPC), np.float32)
    for p in range(128):
        b1[p, p // CH] = 1.0
    b2 = np.ascontiguousarray(b1.T)
    tri = np.zeros((128, 128), np.float32)
    for q in range(128):
        for p in range(128):
            if q // CH == p // CH and q < p:
                tri[q, p] = 1.0
    onesc = np.ones((128, 1), np.float32)
    ones1r = np.ones((64, 128), np.float32)
    identc = np.eye(128, dtype=np.float32)
    lg16c = (np.arange(128)[:, None] * 256 + np.arange(256)[None, :]).astype(np.uint16)
    roff4c = (np.arange(RPC)[:, None] * L).astype(np.float32)
    col = np.arange(16)
    roff16c = np.tile((512.0 * ((col // 4) % 2)).astype(np.float32), (128, 1))
    return b1, b2, tri, onesc, ones1r, identc, lg16c, roff4c, roff16c


def kernel(reps, probs, mask, topk):
    assert topk == K
    from concourse.bass_utils import run_bass_kernel_spmd

    nc = _get_compiled()
    gumbel = _gumbel_const()
    b1, b2, tri, onesc, ones1r, identc, lg16c, roff4c, roff16c = _consts()

    reps = np.ascontiguousarray(reps, dtype=np.float32)
    probs = np.ascontiguousarray(probs, dtype=np.float32)
    mask = np.ascontiguousarray(mask, dtype=np.float32)

    in_maps = []
    for c in range(NCORES):
        sl = slice(c * RPC, (c + 1) * RPC)
        in_maps.append(
            {
                "probs": probs[sl],
                "mask": mask[sl],
                "gumbel": gumbel[sl],
                "reps": reps[sl],
                "b1c": b1,
                "b2c": b2,
                "tric": tri,
                "onesc": onesc,
                "ones1r": ones1r,
                "identc": identc,
                "lg16c": lg16c,
                "roff4c": roff4c,
                "roff16c": roff16c,
            }
        )

    trace = os.environ.get("KTRACE", "") == "1"
    res = run_bass_kernel_spmd(
        nc, in_maps, core_ids=list(range(NCORES)), trace=trace,
        tmpdir=os.environ.get("KTRACE_DIR") or None,
    )
    if trace:
        print(f"HW exec time: {res.exec_time_ns} ns")
    outs = res.results

    sampled = np.concatenate([o["out_reps"] for o in outs], axis=0)
    ind = np.concatenate([o["out_ind"] for o in outs], axis=0)
    dbg = np.concatenate([o["out_dbg"] for o in outs], axis=0)
    counts = dbg[:, 0]
    if not np.all(counts == float(K)):
        raise RuntimeError(f"bisection failed: per-row counts {counts}")
    return sampled, ind.astype(np.int32)


if __name__ == "__main__":
    rng = np.random.default_rng(0)
    reps = rng.standard_normal((B, L, D)).astype(np.float32)
    probs = rng.standard_normal((B, L)).astype(np.float32)
    mask = rng.integers(0, 2, (B, L)).astype(np.float32)
    s, i = kernel(reps=reps, probs=probs, mask=mask, topk=K)
    print(s.shape, i.shape, i[:2, :8])
